# revision 40
# baseline (speedup 1.0000x reference)
"""Trainium2 Bass kernel for nn_MultiHeadAttention_84052509983469.

Full-input contract: kernel(**inputs) takes the complete tensors and
returns the complete [B, S, D] output. Work is sharded across 8 cores as
(batch b in {0,1}) x (head-group g in {0..3}): tensor-parallel over heads
(4 heads / 256 features per core), data-parallel over batch.

Per-core dataflow (all SBUF tensors bf16, PSUM f32):

  K^T,Q^T = W{k,q}_g @ x_b^T      head dims pre-permuted on host to
                                  [e0-15 | o0-15 | e16-31 | o16-31] per
                                  head so the RoPE partner swap is an
                                  intra-32-partition stream_shuffle
  RoPE    : out = x*ct + shuffle(x*st'), st' = [+s;-s] blocks (signs baked)
  V       = x_b @ Wv_g^T          gated by the pad mask + gated ones col
                                  (softmax denominator accumulates in PSUM)
  S^T     = K_h^T.T @ Q_h^T       scores transposed: keys on partitions;
                                  causal diagonal split: tile 2c full 256
                                  cols, tile 2c+1 only the upper 128 cols
  P^T     = exp(S^T / 8) bf16     (no max subtraction; scores are small);
                                  diagonal region masked multiplicatively
  O^T     = V_aug^T @ P^T         row 64 = denominator
  attn^T  = O^T[0:64] * bcast(1/O^T[64])
  partial = attn^T.T @ Wo_g^T     row-sharded Wo, evicted bf16 via gpsimd

Host gathers: out[b] = sum_g partial[b,g] + bo + bv @ Wo^T.
"""

import sys

if "/opt/trn_rl_repo" not in sys.path:
    sys.path.insert(0, "/opt/trn_rl_repo")

import numpy as np

import concourse.bass as bass
import concourse.mybir as mybir
import concourse.tile as tile
from concourse import bacc

# Problem shapes
B, S, D, H = 2, 2048, 1024, 16
HD = D // H  # 64
G = 4  # head groups (cores per batch)
HL = H // G  # heads per core = 4
GF = HL * HD  # features per core = 256
P = 128
NK = S // P  # 16 key tiles
NQ = 256  # query chunk size
NQC = S // NQ  # 8 query chunks
KT = D // P  # 8 contraction tiles for projections
DW = 384  # diag region width: 256 (tile 2c) + 128 (tile 2c+1 upper half)

F32 = mybir.dt.float32
BF16 = mybir.dt.bfloat16
SHUF16 = [i ^ 16 for i in range(32)]  # RoPE partner swap within quadrants


def build_nc(repeat=1, dbg=False):
    nc = bacc.Bacc(None, target_bir_lowering=False, debug=False)

    xt = nc.dram_tensor("xt", [P, KT, S], BF16, kind="ExternalInput")  # x^T tiles
    wq = nc.dram_tensor("wq", [P, KT, GF], BF16, kind="ExternalInput")
    wk = nc.dram_tensor("wk", [P, KT, GF], BF16, kind="ExternalInput")
    wv = nc.dram_tensor("wv", [P, KT, GF], BF16, kind="ExternalInput")
    wo = nc.dram_tensor("wo", [P, 2, D], BF16, kind="ExternalInput")  # Wo^T rows
    bq = nc.dram_tensor("bq", [P, 2], F32, kind="ExternalInput")
    bk = nc.dram_tensor("bk", [P, 2], F32, kind="ExternalInput")
    ct = nc.dram_tensor("ct", [P, S], BF16, kind="ExternalInput")  # cos table
    st = nc.dram_tensor("st", [P, S], BF16, kind="ExternalInput")  # +-sin table
    gate = nc.dram_tensor("gate", [P, NK, HL], F32, kind="ExternalInput")
    cm = nc.dram_tensor("cm", [P, DW], BF16, kind="ExternalInput")  # diag masks
    out = nc.dram_tensor("out", [P, NK, D], BF16, kind="ExternalOutput")
    if dbg:
        d_qt = nc.dram_tensor("d_qt", [P, 2, S], BF16, kind="ExternalOutput")
        d_kt = nc.dram_tensor("d_kt", [P, 2, S], BF16, kind="ExternalOutput")
        d_va = nc.dram_tensor("d_va", [P, NK, HL, HD + 1], BF16, kind="ExternalOutput")
        d_at = nc.dram_tensor("d_at", [P, 2, S], BF16, kind="ExternalOutput")

    with tile.TileContext(nc) as tc:
        with (
            tc.tile_pool(name="xtp", bufs=1) as xtp,
            tc.tile_pool(name="wp", bufs=1) as wp,
            tc.tile_pool(name="const", bufs=1) as constp,
            tc.tile_pool(name="qk", bufs=1) as qkp,
            tc.tile_pool(name="rope", bufs=2) as ropep,
            tc.tile_pool(name="vaug", bufs=1) as vaugp,
            tc.tile_pool(name="pe", bufs=3) as pep,
            tc.tile_pool(name="att", bufs=1) as attp,
            tc.tile_pool(name="nrm", bufs=2) as nrmp,
            tc.tile_pool(name="ob", bufs=3) as obp,
            tc.tile_pool(name="mm", bufs=2, space="PSUM") as mmp,
            tc.tile_pool(name="sc", bufs=2, space="PSUM") as scp,
            tc.tile_pool(name="pop", bufs=1, space="PSUM") as pop,
        ):
            for _rep in range(repeat):
                # ---- input DMAs, ordered by first use ----
                xt_sb = xtp.tile([P, KT, S], BF16, tag="xt")
                wq_sb = wp.tile([P, KT, GF], BF16, tag="wq")
                wk_sb = wp.tile([P, KT, GF], BF16, tag="wk")
                wv_sb = wp.tile([P, KT, GF], BF16, tag="wv")
                wo_sb = wp.tile([P, 2, D], BF16, tag="wo")
                bq_sb = constp.tile([P, 2], F32, tag="bq")
                bk_sb = constp.tile([P, 2], F32, tag="bk")
                ct_sb = constp.tile([P, S], BF16, tag="ct")
                st_sb = constp.tile([P, S], BF16, tag="st")
                gate_sb = constp.tile([P, NK, HL], F32, tag="gate")
                cm_sb = constp.tile([P, DW], BF16, tag="cm")

                # Act warmup: a dummy activation at t~0 pulls the activation
                # function-table load (1.3us) into the initial DMA wait
                # instead of blocking the first PSUM eviction.
                scr = constp.tile([1, 4], F32, tag="scr", name="scr")
                nc.vector.memset(scr[:], 0.0)
                nc.scalar.activation(
                    out=scr[:], in_=scr[:], func=mybir.ActivationFunctionType.Exp
                )

                # bulk DMAs on one queue, ordered by first-need time; the DMA
                # device serves them in this order. xt chunk 0 split by k-tile
                # halves and ct/st split by columns so the first proj chunk's
                # dependency prefix is as short as possible.
                nc.sync.dma_start(out=wk_sb[:], in_=wk[:])
                nc.sync.dma_start(out=xt_sb[:, 0:4, 0:512], in_=xt[:, 0:4, 0:512])
                nc.sync.dma_start(out=xt_sb[:, 4:8, 0:512], in_=xt[:, 4:8, 0:512])
                nc.sync.dma_start(out=wq_sb[:], in_=wq[:])
                nc.sync.dma_start(out=bk_sb[:], in_=bk[:])
                nc.sync.dma_start(out=bq_sb[:], in_=bq[:])
                nc.sync.dma_start(out=wv_sb[:], in_=wv[:])
                nc.sync.dma_start(out=ct_sb[:, 0:512], in_=ct[:, 0:512])
                nc.sync.dma_start(out=st_sb[:, 0:512], in_=st[:, 0:512])
                nc.sync.dma_start(out=gate_sb[:], in_=gate[:])
                nc.sync.dma_start(out=cm_sb[:], in_=cm[:])
                nc.sync.dma_start(out=ct_sb[:, 512:S], in_=ct[:, 512:S])
                nc.sync.dma_start(out=xt_sb[:, :, 512:1024], in_=xt[:, :, 512:1024])
                nc.sync.dma_start(out=st_sb[:, 512:S], in_=st[:, 512:S])
                nc.sync.dma_start(out=wo_sb[:], in_=wo[:])
                nc.sync.dma_start(out=xt_sb[:, :, 1024:1536], in_=xt[:, :, 1024:1536])
                nc.sync.dma_start(out=xt_sb[:, :, 1536:2048], in_=xt[:, :, 1536:2048])

                # ---- K^T / Q^T projection + bias + RoPE for one 512-col chunk ----
                qt = [qkp.tile([P, S], BF16, tag=f"qt{m}", name=f"qt{m}") for m in range(2)]
                kt_ = [qkp.tile([P, S], BF16, tag=f"kt{m}", name=f"kt{m}") for m in range(2)]

                def emit_proj_chunk(c4):
                    # m-major: head-pair 0's K/Q rope first, so pair-0 scores
                    # of the following attention chunk unblock sooner
                    cs = slice(c4 * 512, (c4 + 1) * 512)
                    for m in range(2):
                        for dst, w_sb, b_sb in ((kt_, wk_sb, bk_sb), (qt, wq_sb, bq_sb)):
                            ps = mmp.tile([P, 512], F32, tag="acc", name="ps_proj")
                            for k in range(KT):
                                nc.tensor.matmul(
                                    ps[:],
                                    w_sb[:, k, m * P : (m + 1) * P],
                                    xt_sb[:, k, cs],
                                    start=(k == 0),
                                    stop=(k == KT - 1),
                                )
                            nc.scalar.activation(
                                out=dst[m][:, cs],
                                in_=ps[:],
                                func=mybir.ActivationFunctionType.Identity,
                                bias=b_sb[:, m : m + 1],
                                scale=1.0,
                            )
                            ra = ropep.tile([P, 512], BF16, tag="ra", name="ra")
                            rb_ = ropep.tile([P, 512], BF16, tag="rb", name="rb")
                            rs = ropep.tile([P, 512], BF16, tag="rs", name="rs")
                            nc.vector.tensor_mul(ra[:], dst[m][:, cs], ct_sb[:, cs])
                            nc.vector.tensor_mul(rb_[:], dst[m][:, cs], st_sb[:, cs])
                            nc.vector.stream_shuffle(rs[:], rb_[:], SHUF16)
                            nc.vector.tensor_add(dst[m][:, cs], ra[:], rs[:])

                # ---- V projection (emitted as PE filler during attention) ----
                vaug = [
                    vaugp.tile([P, HL, HD + 1], BF16, tag=f"vaug{t}", name=f"vaug{t}")
                    for t in range(NK)
                ]

                def emit_vproj(t):
                    ps = mmp.tile(
                        [P, GF], F32, tag="acc", name="ps_v", padded_shape=[P, 512]
                    )
                    for k in range(KT):
                        nc.tensor.matmul(
                            ps[:],
                            xt_sb[:, k, t * P : (t + 1) * P],
                            wv_sb[:, k, :],
                            start=(k == 0),
                            stop=(k == KT - 1),
                        )
                    # gated eviction: early tiles on Act (DVE is rope-heavy
                    # during chunks 0-3), late tiles on DVE (Act is exp-bound)
                    if t < 8:
                        nc.scalar.activation(
                            out=vaug[t][:, :, 0:HD],
                            in_=ps.rearrange("p (h d) -> p h d", h=HL),
                            func=mybir.ActivationFunctionType.Copy,
                            scale=gate_sb[:, t, 0:1],
                        )
                        nc.scalar.activation(
                            out=vaug[t][:, :, HD : HD + 1],
                            in_=gate_sb[:, t, :].unsqueeze(-1),
                            func=mybir.ActivationFunctionType.Copy,
                        )
                    else:
                        nc.vector.tensor_scalar_mul(
                            vaug[t][:, :, 0:HD],
                            ps.rearrange("p (h d) -> p h d", h=HL),
                            gate_sb[:, t, 0:1],
                        )
                        nc.vector.tensor_copy(
                            out=vaug[t][:, :, HD : HD + 1],
                            in_=gate_sb[:, t, :].unsqueeze(-1),
                        )

                # ---- output projection (also PE filler) ----
                attnt = [
                    attp.tile([P, S], BF16, tag=f"at{m}", name=f"at{m}") for m in range(2)
                ]

                def emit_outproj(t):
                    ob = obp.tile([P, D], BF16, tag="ob", name="ob")
                    for oc in range(2):
                        ps = mmp.tile([P, 512], F32, tag="acc", name="ps_o")
                        for m in range(2):
                            nc.tensor.matmul(
                                ps[:],
                                attnt[m][:, t * P : (t + 1) * P],
                                wo_sb[:, m, oc * 512 : (oc + 1) * 512],
                                start=(m == 0),
                                stop=(m == 1),
                            )
                        oslice = ob[:, oc * 512 : (oc + 1) * 512]
                        if t >= 2 * NQC - 2:
                            # tail tiles: t=15 evicts on Act (idle after the
                            # final exp) in parallel with t=14 on DVE, and
                            # stores split per-oc so they overlap the evicts
                            if t == 2 * NQC - 1:
                                nc.scalar.activation(
                                    out=oslice, in_=ps[:],
                                    func=mybir.ActivationFunctionType.Identity,
                                )
                            else:
                                nc.vector.tensor_copy(out=oslice, in_=ps[:])
                            nc.sync.dma_start(
                                out=out[:, t, oc * 512 : (oc + 1) * 512], in_=oslice
                            )
                        else:
                            nc.vector.tensor_copy(out=oslice, in_=ps[:])
                    if t < 2 * NQC - 2:
                        nc.sync.dma_start(out=out[:, t, :], in_=ob[:])

                # filler queue: (vtile_or_None, fn) emitting ~0.5us of
                # independent PE work, interleaved into the attention stream
                # to absorb the exp latency. V items must be emitted before
                # the chunk that consumes them (forced drain below).
                fillers = []
                for t in range(4):
                    fillers.append((t, lambda t=t: emit_vproj(t)))

                def pop_filler(n=1):
                    for _ in range(n):
                        if fillers:
                            fillers.pop(0)[1]()

                def force_vtiles(tmax):
                    while any(v is not None and v <= tmax for v, _ in fillers):
                        pop_filler(1)

                # ---- attention: one chunk of 256 queries, head-pairs in
                # lockstep, scores/PV pipelined against the exp on Act ----
                def emit_attn_chunk(c):
                    q0 = c * NQ
                    force_vtiles(2 * c + 1)
                    # group layout for this chunk: full tiles in 4s; attach the
                    # diagonal pair to the last group when it has <= 2 tiles
                    fulls = list(range(2 * c))
                    glist = [fulls[i : i + 4] for i in range(0, len(fulls), 4)]
                    if glist and len(glist[-1]) <= 2:
                        glist[-1] = (glist[-1], True)
                        glist[:-1] = [(g, False) for g in glist[:-1]]
                    else:
                        glist = [(g, False) for g in glist] + [([], True)]


                    for pair in range(2):
                        m = pair
                        po = pop.tile(
                            [HD + 1, 2, NQ], F32, tag=f"po{pair}", name=f"po{pair}"
                        )
                        # start=True marks the tile's whole PSUM bank pending-
                        # zero; each byte's first touch then writes fresh and
                        # later touches accumulate. Exactly one start per bank
                        # per chunk (the first PV matmul, i.e. hh=0's first).
                        bank_started = False
                        for g, has_diag in glist:
                            width = len(g) * NQ + (DW if has_diag else 0)
                            pes = []
                            for hh in range(2):
                                h = 2 * pair + hh
                                r0 = 64 * hh
                                ps = scp.tile(
                                    [P, width], F32, tag="ps", name="ps_s",
                                    padded_shape=[P, 1024],
                                )
                                for u, t in enumerate(g):
                                    nc.tensor.matmul(
                                        ps[:, u * NQ : (u + 1) * NQ],
                                        kt_[m][r0 : r0 + 64, t * P : (t + 1) * P],
                                        qt[m][r0 : r0 + 64, q0 : q0 + NQ],
                                        start=True,
                                        stop=True,
                                    )
                                if has_diag:
                                    od = len(g) * NQ
                                    nc.tensor.matmul(
                                        ps[:, od : od + NQ],
                                        kt_[m][r0 : r0 + 64, 2 * c * P : (2 * c + 1) * P],
                                        qt[m][r0 : r0 + 64, q0 : q0 + NQ],
                                        start=True,
                                        stop=True,
                                    )
                                    nc.tensor.matmul(
                                        ps[:, od + NQ : od + DW],
                                        kt_[m][
                                            r0 : r0 + 64,
                                            (2 * c + 1) * P : (2 * c + 2) * P,
                                        ],
                                        qt[m][r0 : r0 + 64, q0 + 128 : q0 + NQ],
                                        start=True,
                                        stop=True,
                                    )
                                pe = pep.tile(
                                    [P, width], BF16, tag="pe", name="pe",
                                    padded_shape=[P, 1024],
                                )
                                nc.scalar.activation(
                                    out=pe[:], in_=ps[:],
                                    func=mybir.ActivationFunctionType.Exp,
                                    scale=0.125,
                                )
                                if has_diag:
                                    od = len(g) * NQ
                                    nc.vector.tensor_mul(
                                        pe[:, od : od + DW], pe[:, od : od + DW], cm_sb[:]
                                    )
                                pes.append(pe)
                            # filler supply is finite: save it for the late
                            # chunks where exp (Act) outpaces scores+PV (PE);
                            # hold some back at c=6 so c=7 doesn't starve
                            pop_filler({0: 0, 1: 0, 2: 1, 3: 1, 6: 1}.get(c, 2))
                            for hh in range(2):
                                h = 2 * pair + hh
                                pe = pes[hh]
                                for u, t in enumerate(g):
                                    nc.tensor.matmul(
                                        po[:, hh, :],
                                        vaug[t][:, h, :],
                                        pe[:, u * NQ : (u + 1) * NQ],
                                        start=(not bank_started),
                                        stop=False,
                                        skip_group_check=True,
                                    )
                                    bank_started = True
                                if has_diag:
                                    od = len(g) * NQ
                                    nc.tensor.matmul(
                                        po[:, hh, :],
                                        vaug[2 * c][:, h, :],
                                        pe[:, od : od + NQ],
                                        start=(not bank_started),
                                        stop=False,
                                        skip_group_check=True,
                                    )
                                    bank_started = True
                                    nc.tensor.matmul(
                                        po[:, hh, 128:NQ],
                                        vaug[2 * c + 1][:, h, :],
                                        pe[:, od + NQ : od + DW],
                                        start=False,
                                        stop=True,
                                        skip_group_check=True,
                                    )
                        # normalize the pair -> attn^T (bf16). Both recips
                        # first so hh1's recip isn't queued behind hh0's TT
                        # (which waits on the Pool broadcast round-trip).
                        rcs, rbs = [], []
                        for hh in range(2):
                            rc = nrmp.tile([1, NQ], F32, tag=f"rc{hh}", name="rc")
                            nc.vector.reciprocal(rc[:], po[HD : HD + 1, hh, :])
                            rcs.append(rc)
                        for hh in range(2):
                            rb = nrmp.tile([64, NQ], F32, tag=f"rbb{hh}", name="rbb")
                            nc.gpsimd.partition_broadcast(rb[:], rcs[hh][0:1, :], channels=64)
                            rbs.append(rb)
                        for hh in range(2):
                            nc.vector.tensor_mul(
                                attnt[m][64 * hh : 64 * hh + 64, q0 : q0 + NQ],
                                po[0:HD, hh, :],
                                rbs[hh][:],
                            )
                    # queue upcoming V tiles and this chunk's out-proj
                    if c < NQC - 2:
                        for t in (2 * c + 4, 2 * c + 5):
                            fillers.append((t, lambda t=t: emit_vproj(t)))
                    for t in (2 * c, 2 * c + 1):
                        fillers.append((None, lambda t=t: emit_outproj(t)))

                # ---- interleaved schedule: proj chunk c4 feeds attention
                # chunks 2*c4, 2*c4+1; attention exp latency is absorbed by
                # the projection / V / out-proj matmuls around it ----
                for c4 in range(S // 512):
                    emit_proj_chunk(c4)
                    emit_attn_chunk(2 * c4)
                    emit_attn_chunk(2 * c4 + 1)
                pop_filler(len(fillers))
                if dbg:
                    for m in range(2):
                        nc.sync.dma_start(out=d_qt[:, m, :], in_=qt[m][:])
                        nc.sync.dma_start(out=d_kt[:, m, :], in_=kt_[m][:])
                        nc.sync.dma_start(out=d_at[:, m, :], in_=attnt[m][:])
                    for t in range(NK):
                        nc.sync.dma_start(out=d_va[:, t, :, :], in_=vaug[t][:])
    nc.compile()
    return nc


# ---------------- host-side prep ----------------

# per-head row layout: [even dims 0-15 | odd 0-15 | even 16-31 | odd 16-31]
_PERM64 = np.concatenate(
    [
        np.arange(0, 32, 2),
        np.arange(1, 32, 2),
        np.arange(32, 64, 2),
        np.arange(33, 64, 2),
    ]
)


def _rope_tables():
    inv = 1.0 / (10000.0 ** (np.arange(0, HD, 2, dtype=np.float32) / HD))
    t = np.arange(S, dtype=np.float32)
    ang = np.outer(t, inv)  # [S, 32]
    return np.cos(ang).astype(np.float32), np.sin(ang).astype(np.float32)


def _tile_rows(a, p=P):
    """[R, N] -> [p, R//p, N] with row r of tile i = a[i*p + r]"""
    R = a.shape[0]
    return np.ascontiguousarray(
        a.reshape(R // p, p, *a.shape[1:]).transpose(1, 0, *range(2, a.ndim + 1))
    )


def _bf16(a):
    import ml_dtypes

    return np.ascontiguousarray(a).astype(ml_dtypes.bfloat16)


def rope_ct_st():
    cos, sin = _rope_tables()  # [S, 32]
    c, s = cos.T, sin.T  # [32, S]
    blk_c = np.concatenate([c[0:16], c[0:16], c[16:32], c[16:32]])  # [64, S]
    blk_s = np.concatenate([s[0:16], -s[0:16], s[16:32], -s[16:32]])
    return np.tile(blk_c, (2, 1)), np.tile(blk_s, (2, 1))  # [128, S]


def shard_inputs(x, effective_len, Wq, bq, Wk, bk, Wv, bv, Wo, bo):
    x = np.asarray(x, np.float32)
    effective_len = np.asarray(effective_len, np.int32)
    Wq, Wk, Wv, Wo = (np.asarray(w, np.float32) for w in (Wq, Wk, Wv, Wo))
    bq, bk = (np.asarray(b, np.float32) for b in (bq, bk))

    ctt, stt = rope_ct_st()
    ctb, stb = _bf16(ctt), _bf16(stt)

    # diag masks: [128, 384] = [tile 2c full 256 cols | tile 2c+1 upper 128]
    kl = np.arange(P)[:, None]
    cmA = (np.arange(NQ)[None, :] >= kl).astype(np.float32)  # [128, 256]
    cmB = (np.arange(128)[None, :] >= kl).astype(np.float32)  # [128, 128]
    cmb = _bf16(np.concatenate([cmA, cmB], axis=1))

    in_maps = []
    for b in range(B):
        xtb = _bf16(_tile_rows(np.ascontiguousarray(x[b].T)))  # [128, 8, S]
        g_vec = (np.arange(S) < (S - int(effective_len[b]))).astype(np.float32)
        gateb = np.ascontiguousarray(
            np.repeat(g_vec.reshape(NK, P).T[:, :, None], HL, axis=2)
        )  # [128, NK, HL]
        for g in range(G):
            rows = np.concatenate(
                [g * GF + h * HD + _PERM64 for h in range(HL)]
            )
            vrows = np.arange(g * GF, (g + 1) * GF)
            in_maps.append(
                {
                    "xt": xtb,
                    "wq": _bf16(_tile_rows(np.ascontiguousarray(Wq[rows].T))),
                    "wk": _bf16(_tile_rows(np.ascontiguousarray(Wk[rows].T))),
                    "wv": _bf16(_tile_rows(np.ascontiguousarray(Wv[vrows].T))),
                    "wo": _bf16(_tile_rows(np.ascontiguousarray(Wo[:, vrows].T))),
                    "bq": np.ascontiguousarray(bq[rows].reshape(2, P).T),
                    "bk": np.ascontiguousarray(bk[rows].reshape(2, P).T),
                    "ct": ctb,
                    "st": stb,
                    "gate": gateb,
                    "cm": cmb,
                }
            )
    return in_maps


def gather_outputs(results, bo, bv_wo=0.0):
    bo = np.asarray(bo, np.float32)
    out = np.zeros((B, S, D), np.float32)
    for b in range(B):
        acc = np.zeros((S, D), np.float32)
        for g in range(G):
            o3 = np.asarray(results[b * G + g]["out"]).astype(np.float32)
            acc += o3.transpose(1, 0, 2).reshape(S, D)
        out[b] = acc + bo + bv_wo
    return out


_NC_CACHE = None


def _get_nc():
    global _NC_CACHE
    if _NC_CACHE is None:
        _NC_CACHE = build_nc()
    return _NC_CACHE


def kernel(**inputs):
    from concourse.bass_utils import run_bass_kernel_spmd

    nc = _get_nc()
    in_maps = shard_inputs(**inputs)
    res = run_bass_kernel_spmd(nc, in_maps, core_ids=list(range(8)))
    bv_wo = np.asarray(inputs["bv"], np.float32) @ np.asarray(
        inputs["Wo"], np.float32
    ).T
    return gather_outputs(res.results, inputs["bo"], bv_wo)
